# revision 3
# baseline (speedup 1.0000x reference)
"""Self-contained TRN2 Bass kernel for the VQ-codebook forward problem.

kernel(z, Wq, bq, emb, Wp, bp) -> (out, idx, loss)
  z    [16, 1024, 512] f32     Wq [512, 256]  bq [256]
  emb  [8192, 256]             Wp [256, 512]  bp [512]
  out  [16, 1024, 512] f32     idx [16384] i32     loss [16, 1024] f32

Strategy: data-parallel over the 16384 tokens across 8 NeuronCores (2048
tokens/core); codebook + weights replicated.  Per core:
  zl = z @ Wq + bq (exact fp32, computed in transposed layout),
  fp16 PE matmul of zl against the l2-normalized codebook -> approximate
  scores, top-8 per token via the DVE Max/MaxIndex sorter, top-4 kept,
  exact fp32 rescore of the 4 candidates (gathered codebook rows) -> idx,
  zq = emb[idx] gather, straight-through zq_st = zl + (zq - zl),
  loss = 1.25 * mean((zq - zl)^2), out = zq_st @ Wp + bp (exact fp32).
"""
import sys

sys.path.insert(0, "/opt/trn_rl_repo")

from contextlib import ExitStack

import numpy as np
import orjson

import concourse.bass as bass
import concourse.mybir as mybir
from concourse.tile import TileContext
from concourse.masks import make_identity
from concourse.bass_utils import run_bass_kernel_spmd

# ---------------------------------------------------------------------------
# Compat shim: this container's walrus build accepts only ONE sync-wait
# command on NO_STRUCT instructions (Drain/NoOp), but the Tile epilogue
# attaches all end-of-kernel waits to a single Drain.  Hoist excess waits
# onto dedicated single-wait NoOps (same engine => program order keeps the
# stall semantics).
_WAIT_CAP = 1  # observed walrus limit: one sync-wait per instruction


def _fix_module_json(raw: bytes) -> bytes:
    mod = orjson.loads(raw)
    ctr = 0
    changed = False
    for fn in mod.get("functions", []):
        for blk in fn.get("blocks", []):
            insts = blk.get("instructions")
            if not insts:
                continue
            out = []
            for ins in insts:
                si = ins.get("sync_info")
                waits = (si or {}).get("on_wait") or []
                cap = _WAIT_CAP
                if len(waits) > cap:
                    for w in waits[:-cap]:
                        ctr += 1
                        out.append({
                            "debug": ins.get("debug", 0),
                            "engine": ins["engine"],
                            "ins": [],
                            "outs": [],
                            "name": f"{ins['name']}-wsplit{ctr}",
                            "opcode": "NoOp",
                            "sync_info": {"on_update": [], "on_wait": [w]},
                        })
                    si["on_wait"] = waits[-cap:]
                    changed = True
                out.append(ins)
            blk["instructions"] = out
    return orjson.dumps(mod) if changed else raw


if not getattr(bass.Bass, "_vq_wait_shim", False):
    _orig_to_json_bytes = bass.Bass.to_json_bytes

    def _patched_to_json_bytes(self) -> bytes:
        return _fix_module_json(_orig_to_json_bytes(self))

    bass.Bass.to_json_bytes = _patched_to_json_bytes
    bass.Bass._vq_wait_shim = True

# ---------------------------------------------------------------------------
F32 = mybir.dt.float32
F16 = mybir.dt.float16
I32 = mybir.dt.int32
U32 = mybir.dt.uint32
ALU = mybir.AluOpType
ACTF = mybir.ActivationFunctionType

D_IN = 512
D = 256
K = 8192
D_OUT = 512
N_CORES = 8
NTOK = 2048                 # tokens per core
NBLK = NTOK // 128
KT = K // 128
SCORE_NT = 1024
N_SC = K // SCORE_NT
NCAND = 4
BETA = 0.25


def build_kernel(nc: bass.Bass):
    zT = nc.declare_dram_parameter("zT", [D_IN, NTOK], F32, isOutput=False)
    Wq = nc.declare_dram_parameter("Wq", [D_IN, D], F32, isOutput=False)
    bq = nc.declare_dram_parameter("bq", [D, 1], F32, isOutput=False)
    emb = nc.declare_dram_parameter("emb", [K, D], F32, isOutput=False)
    Wp = nc.declare_dram_parameter("Wp", [D, D_OUT], F32, isOutput=False)
    bp_b = nc.declare_dram_parameter("bp_b", [128, D_OUT], F32, isOutput=False)

    out_d = nc.declare_dram_parameter("out", [NTOK, D_OUT], F32, isOutput=True)
    idx_d = nc.declare_dram_parameter("idx", [NBLK, 128], I32, isOutput=True)
    loss_d = nc.declare_dram_parameter("loss", [NBLK, 128], F32, isOutput=True)

    with TileContext(nc) as tc, ExitStack() as ctx:
        cpool = ctx.enter_context(tc.tile_pool(name="const", bufs=1))
        ps_s = ctx.enter_context(tc.tile_pool(name="ps_s", bufs=2, space="PSUM"))
        ps_t = ctx.enter_context(tc.tile_pool(name="ps_t", bufs=1, space="PSUM"))
        ps_o = ctx.enter_context(tc.tile_pool(name="ps_o", bufs=1, space="PSUM"))
        p_emb = ctx.enter_context(tc.tile_pool(name="p_emb", bufs=3))
        p_s16 = ctx.enter_context(tc.tile_pool(name="p_s16", bufs=2))
        p_sm = ctx.enter_context(tc.tile_pool(name="p_sm", bufs=3))
        p_scr = ctx.enter_context(tc.tile_pool(name="p_scr", bufs=4))
        p_ej = ctx.enter_context(tc.tile_pool(name="p_ej", bufs=8))
        p_io = ctx.enter_context(tc.tile_pool(name="p_io", bufs=2))

        ident = cpool.tile([128, 128], F32, tag="ident")
        make_identity(nc, ident[:])
        iota4 = cpool.tile([128, NCAND], I32, tag="iota4")
        nc.gpsimd.iota(iota4[:], pattern=[[1, NCAND]], base=0, channel_multiplier=0)
        iota4f = cpool.tile([128, NCAND], F32, tag="iota4f")
        nc.vector.tensor_copy(iota4f[:], iota4[:])

        wq_t = []
        for k in range(4):
            t = cpool.tile([128, D], F32, tag=f"wq{k}", name=f"wq{k}")
            nc.sync.dma_start(t[:], Wq[k * 128:(k + 1) * 128, :])
            wq_t.append(t)
        wp_t = []
        for k in range(2):
            t = cpool.tile([128, D_OUT], F32, tag=f"wp{k}", name=f"wp{k}")
            nc.sync.dma_start(t[:], Wp[k * 128:(k + 1) * 128, :])
            wp_t.append(t)
        bq_t = []
        for m in range(2):
            t = cpool.tile([128, 1], F32, tag=f"bq{m}", name=f"bq{m}")
            nc.sync.dma_start(t[:], bq[m * 128:(m + 1) * 128, :])
            bq_t.append(t)
        bp_t = cpool.tile([128, D_OUT], F32, tag="bp")
        nc.sync.dma_start(bp_t[:], bp_b[:])
        zT_t = []
        for k in range(4):
            t = cpool.tile([128, NTOK], F32, tag=f"zT{k}", name=f"zT{k}")
            nc.sync.dma_start(t[:], zT[k * 128:(k + 1) * 128, :])
            zT_t.append(t)

        # en-prep: normalized codebook, transposed, fp16: enT16[h] [128(d), 8192]
        enT16 = [cpool.tile([128, K], F16, tag=f"enT16_{h}", name=f"enT16_{h}")
                 for h in range(2)]
        for t in range(KT):
            E = p_emb.tile([128, D], F32, tag="E")
            nc.sync.dma_start(E[:], emb[t * 128:(t + 1) * 128, :])
            scr = p_scr.tile([128, D], F32, tag="scr")
            nsq = p_sm.tile([128, 1], F32, tag="nsq")
            nc.vector.scalar_tensor_tensor(
                out=scr[:], in0=E[:], scalar=1.0, in1=E[:],
                op0=ALU.mult, op1=ALU.mult, accum_out=nsq[:])
            norm = p_sm.tile([128, 1], F32, tag="norm")
            nc.scalar.activation(norm[:], nsq[:], ACTF.Sqrt)
            rinv = p_sm.tile([128, 1], F32, tag="rinv")
            nc.vector.reciprocal(rinv[:], norm[:])
            en = p_emb.tile([128, D], F32, tag="en")
            nc.vector.tensor_scalar(
                out=en[:], in0=E[:], scalar1=rinv[:], scalar2=None, op0=ALU.mult)
            for h in range(2):
                ptr = ps_t.tile([128, 128], F32, tag="ptr")
                nc.tensor.transpose(ptr[:], en[:, h * 128:(h + 1) * 128], ident[:])
                nc.scalar.activation(
                    enT16[h][:, t * 128:(t + 1) * 128], ptr[:], ACTF.Copy)

        # zlT = (Wq.T @ zT) + bq, fp32 [2][128(d), 2048]; fp16 copy for phase A
        zlT = [cpool.tile([128, NTOK], F32, tag=f"zlT{m}", name=f"zlT{m}")
               for m in range(2)]
        zlT16 = [cpool.tile([128, NTOK], F16, tag=f"zlT16_{m}", name=f"zlT16_{m}")
                 for m in range(2)]
        for m in range(2):
            for ntile in range(NTOK // 512):
                sl = slice(ntile * 512, (ntile + 1) * 512)
                pzl = ps_o.tile([128, 512], F32, tag="po")
                for k in range(4):
                    nc.tensor.matmul(
                        pzl[:], lhsT=wq_t[k][:, m * 128:(m + 1) * 128],
                        rhs=zT_t[k][:, sl], start=(k == 0), stop=(k == 3))
                nc.vector.tensor_scalar(
                    out=zlT[m][:, sl], in0=pzl[:], scalar1=bq_t[m][:],
                    scalar2=None, op0=ALU.add)
            nc.scalar.activation(zlT16[m][:], zlT[m][:], ACTF.Copy)

        # zl in [token, d] layout per block
        zl_blk = [cpool.tile([128, D], F32, tag=f"zl_blk{b}", name=f"zl_blk{b}")
                  for b in range(NBLK)]
        for b in range(NBLK):
            for m in range(2):
                ptr = ps_t.tile([128, 128], F32, tag="ptr")
                nc.tensor.transpose(
                    ptr[:], zlT[m][:, b * 128:(b + 1) * 128], ident[:])
                nc.vector.tensor_copy(zl_blk[b][:, m * 128:(m + 1) * 128], ptr[:])

        for b in range(NBLK):
            bsl = slice(b * 128, (b + 1) * 128)
            # phase A: fp16 approximate scores [128, 8192]
            s16 = p_s16.tile([128, K], F16, tag="s16")
            for nt in range(N_SC):
                sl = slice(nt * SCORE_NT, (nt + 1) * SCORE_NT)
                ps = ps_s.tile([128, SCORE_NT], F32, tag="ps")
                for half in range(SCORE_NT // 512):
                    hsl = slice(half * 512, (half + 1) * 512)
                    esl = slice(nt * SCORE_NT + half * 512,
                                nt * SCORE_NT + (half + 1) * 512)
                    for kk in range(2):
                        nc.tensor.matmul(
                            ps[:, hsl], lhsT=zlT16[kk][:, bsl],
                            rhs=enT16[kk][:, esl],
                            start=(kk == 0), stop=(kk == 1))
                nc.scalar.activation(s16[:, sl], ps[:], ACTF.Copy)

            max8 = p_sm.tile([128, 8], F16, tag="max8")
            nc.vector.max(max8[:], s16[:])
            idx8 = p_sm.tile([128, 8], U32, tag="idx8")
            nc.vector.max_index(idx8[:], max8[:], s16[:])

            # phase B: exact fp32 rescore of top-4 candidates
            dots = p_sm.tile([128, NCAND], F32, tag="dots")
            nsq4 = p_sm.tile([128, NCAND], F32, tag="nsq4")
            for j in range(NCAND):
                Ej = p_ej.tile([128, D], F32, tag="Ej")
                nc.gpsimd.indirect_dma_start(
                    out=Ej[:], out_offset=None, in_=emb[:],
                    in_offset=bass.IndirectOffsetOnAxis(ap=idx8[:, j:j + 1], axis=0))
                scr = p_scr.tile([128, D], F32, tag="scr")
                nc.vector.scalar_tensor_tensor(
                    out=scr[:], in0=Ej[:], scalar=1.0, in1=zl_blk[b][:],
                    op0=ALU.mult, op1=ALU.mult, accum_out=dots[:, j:j + 1])
                scr2 = p_scr.tile([128, D], F32, tag="scr2")
                nc.vector.scalar_tensor_tensor(
                    out=scr2[:], in0=Ej[:], scalar=1.0, in1=Ej[:],
                    op0=ALU.mult, op1=ALU.mult, accum_out=nsq4[:, j:j + 1])
            norm4 = p_sm.tile([128, NCAND], F32, tag="norm4")
            nc.scalar.activation(norm4[:], nsq4[:], ACTF.Sqrt)
            rinv4 = p_sm.tile([128, NCAND], F32, tag="rinv4")
            nc.vector.reciprocal(rinv4[:], norm4[:])
            sc8 = p_sm.tile([128, 8], F32, tag="sc8")
            nc.gpsimd.memset(sc8[:], -3.0e38)
            nc.vector.tensor_tensor(
                out=sc8[:, 0:NCAND], in0=dots[:], in1=rinv4[:], op=ALU.mult)
            m8b = p_sm.tile([128, 8], F32, tag="m8b")
            nc.vector.max(m8b[:], sc8[:])
            pos8 = p_sm.tile([128, 8], U32, tag="pos8")
            nc.vector.max_index(pos8[:], m8b[:], sc8[:])
            posf = p_sm.tile([128, 1], F32, tag="posf")
            nc.vector.tensor_copy(posf[:], pos8[:, 0:1])
            candf = p_sm.tile([128, NCAND], F32, tag="candf")
            nc.vector.tensor_copy(candf[:], idx8[:, 0:NCAND])
            eq4 = p_sm.tile([128, NCAND], F32, tag="eq4")
            nc.vector.tensor_scalar(
                out=eq4[:], in0=iota4f[:], scalar1=posf[:], scalar2=None,
                op0=ALU.is_equal)
            scr4 = p_scr.tile([128, NCAND], F32, tag="scr4")
            idxf = p_sm.tile([128, 1], F32, tag="idxf")
            nc.vector.scalar_tensor_tensor(
                out=scr4[:], in0=eq4[:], scalar=1.0, in1=candf[:],
                op0=ALU.mult, op1=ALU.mult, accum_out=idxf[:])
            idx_i = p_sm.tile([128, 1], I32, tag="idx_i")
            nc.vector.tensor_copy(idx_i[:], idxf[:])
            nc.sync.dma_start(idx_d[b:b + 1, :], idx_i[:])

            # zq gather, losses, straight-through
            zq = p_io.tile([128, D], F32, tag="zq")
            nc.gpsimd.indirect_dma_start(
                out=zq[:], out_offset=None, in_=emb[:],
                in_offset=bass.IndirectOffsetOnAxis(ap=idx_i[:, 0:1], axis=0))
            tdif = p_io.tile([128, D], F32, tag="tdif")
            nc.gpsimd.tensor_tensor(
                out=tdif[:], in0=zq[:], in1=zl_blk[b][:], op=ALU.subtract)
            zq_st = p_io.tile([128, D], F32, tag="zq_st")
            nc.gpsimd.tensor_tensor(
                out=zq_st[:], in0=zl_blk[b][:], in1=tdif[:], op=ALU.add)
            scr5 = p_scr.tile([128, D], F32, tag="scr5")
            lsum = p_sm.tile([128, 1], F32, tag="lsum")
            nc.vector.scalar_tensor_tensor(
                out=scr5[:], in0=tdif[:], scalar=1.0, in1=tdif[:],
                op0=ALU.mult, op1=ALU.mult, accum_out=lsum[:])
            enc = p_sm.tile([128, 1], F32, tag="enc")
            nc.vector.tensor_scalar(
                out=enc[:], in0=lsum[:], scalar1=1.0 / D, scalar2=None,
                op0=ALU.mult)
            encb = p_sm.tile([128, 1], F32, tag="encb")
            nc.vector.tensor_scalar(
                out=encb[:], in0=enc[:], scalar1=BETA, scalar2=None, op0=ALU.mult)
            lossv = p_sm.tile([128, 1], F32, tag="lossv")
            nc.vector.tensor_tensor(
                out=lossv[:], in0=enc[:], in1=encb[:], op=ALU.add)
            nc.sync.dma_start(loss_d[b:b + 1, :], lossv[:])

            # out = zq_st @ Wp + bp
            zq_stT = p_io.tile([128, D], F32, tag="zq_stT")
            for m in range(2):
                ptr = ps_t.tile([128, 128], F32, tag="ptr")
                nc.tensor.transpose(
                    ptr[:], zq_st[:, m * 128:(m + 1) * 128], ident[:])
                nc.vector.tensor_copy(zq_stT[:, m * 128:(m + 1) * 128], ptr[:])
            po = ps_o.tile([128, D_OUT], F32, tag="po")
            for m in range(2):
                nc.tensor.matmul(
                    po[:], lhsT=zq_stT[:, m * 128:(m + 1) * 128], rhs=wp_t[m][:],
                    start=(m == 0), stop=(m == 1))
            out_sb = p_io.tile([128, D_OUT], F32, tag="out_sb")
            nc.vector.tensor_tensor(
                out=out_sb[:], in0=po[:], in1=bp_t[:], op=ALU.add)
            nc.sync.dma_start(out_d[bsl, :], out_sb[:])

    return nc


_CACHED_NC = None


def _get_nc():
    global _CACHED_NC
    if _CACHED_NC is None:
        nc = bass.Bass("TRN2", target_bir_lowering=False, debug=False,
                       num_devices=N_CORES)
        build_kernel(nc)
        _CACHED_NC = nc
    return _CACHED_NC


def run(z, Wq, bq, emb, Wp, bp, trace=False):
    z = np.ascontiguousarray(np.asarray(z, np.float32)).reshape(-1, D_IN)
    Wq = np.ascontiguousarray(np.asarray(Wq, np.float32))
    bqc = np.ascontiguousarray(np.asarray(bq, np.float32).reshape(D, 1))
    emb = np.ascontiguousarray(np.asarray(emb, np.float32))
    Wp = np.ascontiguousarray(np.asarray(Wp, np.float32))
    bp_b = np.ascontiguousarray(
        np.broadcast_to(np.asarray(bp, np.float32)[None, :], (128, D_OUT)))

    in_maps = []
    for c in range(N_CORES):
        zT = np.ascontiguousarray(z[c * NTOK:(c + 1) * NTOK].T)
        in_maps.append({"zT": zT, "Wq": Wq, "bq": bqc, "emb": emb,
                        "Wp": Wp, "bp_b": bp_b})

    nc = _get_nc()
    res = run_bass_kernel_spmd(nc, in_maps, core_ids=list(range(N_CORES)),
                               trace=trace)
    outs = np.concatenate([res.results[c]["out"] for c in range(N_CORES)], axis=0)
    idxs = np.concatenate(
        [res.results[c]["idx"].reshape(-1) for c in range(N_CORES)], axis=0)
    losss = np.concatenate(
        [res.results[c]["loss"].reshape(-1) for c in range(N_CORES)], axis=0)
    out = outs.reshape(16, 1024, D_OUT)
    loss = losss.reshape(16, 1024)
    return (out, idxs.astype(np.int32), loss), res


def kernel(z, Wq, bq, emb, Wp, bp):
    (out, idx, loss), _ = run(z, Wq, bq, emb, Wp, bp, trace=False)
    return out, idx, loss
